# revision 13
# baseline (speedup 1.0000x reference)
"""Trainium2 Bass kernel for nn_FRAMES_VisionTransformer_28166395527587.

The reference computation (drop CLS token -> 1D nearest resize 768->729 ->
reverse-patching reshape to (144,126,126) -> 3D nearest resize to (64,64,64))
is a pure gather with compile-time-constant index maps:

    out[b, 0, z, y, x] = hs[b, 1 + 196*(z//4) + 14*r(y) + p(x),
                            f[81*d0(z) + 9*d1(y) + d2(x)]]

with  d0(z) = [0,2,4,6][z%4],          i(z) = z//4
      c(y)  = floor(63y/32) = 9*r + d1  (0, then odds 1..63, evens 64..124)
      c(x)  = floor(63x/32) = 9*p + d2  (same map)
      f[j]  = floor32(j*768/729)        (float32 floor, matching jax)

Sharding: pure data parallel, 8 batch samples per core.  The CLS token is
stripped host-side so the (sample, couple) block stride is uniform; on each
core the 128 SBUF partitions then hold the 128 (sample, couple) blocks and
every DMA spans all 128 partitions (all 16 SDMA engines).  The gather runs
as a short sequence of strided on-chip copies shared by all partitions.
"""

import numpy as np

# ---------------------------------------------------------------- constants
B_FULL = 64
N_CORES = 8
B_CORE = B_FULL // N_CORES  # 8 samples per core


def _nearest_f32(out_size, in_size):
    """float32-exact emulation of the reference's jnp _nearest_idx.

    jax computes floor(arange(out) * (in/out)) in float32; at j=486 the
    product rounds to 511.999... so floor gives 511, not the exact 512."""
    ratio = np.float32(in_size / out_size)
    j = np.arange(out_size, dtype=np.int32).astype(np.float32)
    return np.floor((j * ratio).astype(np.float32)).astype(np.int64)


_f = _nearest_f32(729, 768)  # feature resize map
_c = _nearest_f32(64, 126)  # y/x resize map (= 9*r + d1)

DZ = [0, 2, 4, 6]  # d0 values for z%4
LOS = [int(_f[81 * d0]) for d0 in DZ]  # [0, 170, 341, 511]
UW = 96  # features per token: 384 B = 3x128 B aligned descriptors


def _feat_runs(q):
    """Contiguous runs of the 81-feature selection for d0-slice q.

    Returns [(j0, n, u0)]: M[:, j0:j0+n] = L[:, u0:u0+n]."""
    g = _f[81 * DZ[q] + np.arange(81)] - LOS[q]
    runs, start = [], 0
    for k in range(1, 81):
        if g[k] != g[k - 1] + 1:
            runs.append((start, k - start, int(g[start])))
            start = k
    runs.append((start, 81 - start, int(g[start])))
    return runs


FEAT_RUNS = [_feat_runs(q) for q in range(4)]


def _x_runs():
    """x-gather runs: [(p, x0, nx, d20)] with d2 = d20+2k, x = x0+k."""
    runs, x = [], 0
    while x < 64:
        p, d20 = int(_c[x]) // 9, int(_c[x]) % 9
        n = 1
        while x + n < 64 and _c[x + n] == _c[x] + 2 * n and _c[x + n] // 9 == p:
            n += 1
        runs.append((p, x, n, d20))
        x += n
    return runs


X_RUNS = _x_runs()

# ------------------------------------------------------------- bass program
_NC_CACHE = None


def _build_nc():
    import concourse.bacc as bacc
    import concourse.tile as tile
    from concourse import mybir

    nc = bacc.Bacc(None, target_bir_lowering=False, debug=False)
    f32 = mybir.dt.float32

    # CLS token already stripped host-side -> uniform (b, i) block stride.
    hs = nc.dram_tensor("hs", (B_CORE, 3136, 768), f32, kind="ExternalInput")
    out = nc.dram_tensor("out", (B_CORE, 1, 64, 64, 64), f32, kind="ExternalOutput")

    # [(b i), h, t, u]: 128 blocks x row-half x 98 tokens x feature
    hs_v = hs.ap().rearrange("b (i h t) u -> (b i) h t u", i=16, h=2, t=98)
    # [(b i), q, h, (yl x)]: z = 4i+q, y = 32h+yl; yl,x merge (contiguous)
    out_v = out.ap().rearrange(
        "b c (i q) (h yl) x -> (b i) c q h (yl x)", i=16, q=4, h=2, yl=32
    )

    with tile.TileContext(nc) as tc:
        with (
            tc.tile_pool(name="lp", bufs=2) as lp,
            tc.tile_pool(name="mp", bufs=2) as mp,
            tc.tile_pool(name="xp", bufs=1) as xp,
            tc.tile_pool(name="op", bufs=2) as op,
        ):
            for q in range(4):
                for h in range(2):
                    # ---- load: [128 part = (b,i), 98 tokens, UW features]
                    # one DMA, all 16 SDMA engines
                    L = lp.tile([128, 98 * UW], f32, tag="L")
                    lo = LOS[q]
                    nc.sync.dma_start(
                        out=L[:].rearrange("p (t u) -> p t u", u=UW),
                        in_=hs_v[:, h, :, lo : lo + UW],
                    )

                    # ---- feature compaction: [part, t, 81] (ScalarE)
                    M = mp.tile([128, 98 * 81], f32, tag="M")
                    L3 = L[:].rearrange("p (t u) -> p t u", u=UW)
                    M3 = M[:].rearrange("p (t j) -> p t j", j=81)
                    for (j0, n, u0) in FEAT_RUNS[q]:
                        nc.scalar.copy(
                            out=M3[:, :, j0 : j0 + n], in_=L3[:, :, u0 : u0 + n]
                        )

                    # ---- x-gather: [part, rt, d1, x]  (rt = local r, 7 rows)
                    X = xp.tile([128, 7 * 9 * 64], f32, tag="X")
                    M5 = M[:].rearrange(
                        "p (rt pp d1 d2) -> p rt pp d1 d2", rt=7, pp=14, d1=9
                    )
                    X4 = X[:].rearrange("p (rt d1 x) -> p rt d1 x", rt=7, x=64)
                    for (pp, x0, nx, d20) in X_RUNS:
                        nc.vector.tensor_copy(
                            out=X4[:, :, :, x0 : x0 + nx],
                            in_=M5[:, :, pp, :, d20 : d20 + 2 * nx - 1 : 2],
                        )

                    # ---- y-gather: [part, yl, x];  yl=0 <- c=0, yl k -> c=2k-1
                    O = op.tile([128, 32 * 64], f32, tag="O")
                    X3 = X[:].rearrange("p (c x) -> p c x", x=64)
                    O3 = O[:].rearrange("p (yl x) -> p yl x", x=64)
                    nc.vector.tensor_copy(out=O3[:, 0, :], in_=X3[:, 0, :])
                    nc.vector.tensor_copy(out=O3[:, 1:32, :], in_=X3[:, 1:62:2, :])

                    # ---- store: per partition 8 KiB contiguous in HBM.
                    # GpSimd (SWDGE) so the in-order ACT sequencer never
                    # blocks on the store's wait for the y-gather.
                    nc.gpsimd.dma_start(out=out_v[:, 0, q, h, :], in_=O[:])

    nc.compile()
    return nc


def _get_nc():
    global _NC_CACHE
    if _NC_CACHE is None:
        _NC_CACHE = _build_nc()
    return _NC_CACHE


# ------------------------------------------------------------------ runner
def _in_maps(hidden_states: np.ndarray) -> list:
    hs = np.asarray(hidden_states, dtype=np.float32)
    assert hs.shape == (B_FULL, 3137, 768), hs.shape
    return [
        {"hs": np.ascontiguousarray(hs[c * B_CORE : (c + 1) * B_CORE, 1:, :])}
        for c in range(N_CORES)
    ]


def kernel(hidden_states: np.ndarray) -> np.ndarray:
    from concourse import bass_utils

    nc = _get_nc()
    res = bass_utils.run_bass_kernel_spmd(
        nc, _in_maps(hidden_states), core_ids=list(range(N_CORES))
    )
    return np.concatenate([r["out"] for r in res.results], axis=0)
